# revision 31
# baseline (speedup 1.0000x reference)
"""AdaptiveContextRouter Trainium2 kernel (8 NeuronCores, data-parallel).

Per core (2048 tokens): fp32 matmuls for sel/wq scores (PE, bias folded in as a
K=1 ones-row matmul), complexity net, then exact top-256 per token via
max8-leaf extraction + payload-carrying bitonic merge (key, global index and
wq value move together through XOR-swap comparators on the DVE).
"""

import sys
sys.path.insert(0, "/opt/trn_rl_repo")

import numpy as np
import concourse.bass as bass
import concourse.mybir as mybir
from concourse import bacc
from concourse.tile import TileContext
from concourse.tile_rust import add_dep_helper

F32 = mybir.dt.float32
U32 = mybir.dt.uint32
I32 = mybir.dt.int32
AF = mybir.ActivationFunctionType
OP = mybir.AluOpType

NCORES = 8
B, S, D, POOL = 4, 4096, 1024, 4096
K_MIN, K_MAX = 32, 256
TOK = B * S // NCORES          # 2048 tokens per core
P = 128
NT = TOK // P                  # 16 token tiles
KC = D // P                    # 8 contraction chunks
NCH = 512                      # psum chunk width for matmuls
RUN = 32                       # leaf run length (24 real + 8 pad)
NCHUNK = POOL // P             # 32 chunks
W = NCHUNK * RUN               # 1024 merge width

_cache = {}


class ChainedVec:
    """Proxy for nc.vector that chains every emitted DVE instruction in
    program order (the DVE is the serial bottleneck anyway)."""

    def __init__(self, nc):
        self._nc = nc
        self._lastv = None

    def __getattr__(self, name):
        return getattr(self._nc.vector, name)


def rev8(ap):
    """[128, 8] view reversed along free dim."""
    new = [list(x) for x in ap.ap]
    assert new[-1][0] == 1 and new[-1][1] == 8
    new[-1][0] = -1
    return bass.AP(ap.tensor, ap.offset + 7, new)


def strided_view(ap, off, dims):
    """AP at ap.offset+off with partition dim kept and given free [step,count] dims."""
    new = [list(ap.ap[0])] + [list(d) for d in dims]
    return bass.AP(ap.tensor, ap.offset + off, new)


def flat_view(ap, counts):
    """Contiguous AP over `ap`'s start matching a [c0, c1, ...] count structure."""
    dims = []
    stride = 1
    for c in reversed(counts):
        dims.append([stride, c])
        stride *= c
    dims.reverse()
    return bass.AP(ap.tensor, ap.offset, [list(ap.ap[0])] + dims)


def build(nt=NT):
    tok = nt * P
    nc = bacc.Bacc("TRN2", target_bir_lowering=False)

    xT = nc.dram_tensor("xT", (D, tok), F32, kind="ExternalInput")
    selwT = nc.dram_tensor("selwT", (D, POOL), F32, kind="ExternalInput")
    wqwT = nc.dram_tensor("wqwT", (D, POOL), mybir.dt.bfloat16, kind="ExternalInput")
    xbfT = nc.dram_tensor("xbfT", (D, tok), mybir.dt.bfloat16, kind="ExternalInput")
    c1wT = nc.dram_tensor("c1wT", (D, 256), F32, kind="ExternalInput")
    selb = nc.dram_tensor("selb", (1, POOL), F32, kind="ExternalInput")
    wqb = nc.dram_tensor("wqb", (1, POOL), F32, kind="ExternalInput")
    c1b = nc.dram_tensor("c1b", (256, 1), F32, kind="ExternalInput")
    c2wT = nc.dram_tensor("c2wT", (256, 1), F32, kind="ExternalInput")
    c2b = nc.dram_tensor("c2b", (1, 1), F32, kind="ExternalInput")

    scores_o = nc.dram_tensor("scores", (tok, POOL), F32, kind="ExternalOutput")
    idx_o = nc.dram_tensor("idx", (tok, K_MAX), I32, kind="ExternalOutput")
    pw_o = nc.dram_tensor("pw", (tok, K_MAX), F32, kind="ExternalOutput")
    kv_o = nc.dram_tensor("kv", (tok, 1), I32, kind="ExternalOutput")

    wqs_s = nc.dram_tensor("wqs_s", (tok, POOL), mybir.dt.bfloat16, kind="Internal")
    dbg = _cache.get("dbg")
    if dbg:
        dbgK = nc.dram_tensor("dbgK", (tok, 6 * W), F32, kind="ExternalOutput")
        dbgG = nc.dram_tensor("dbgG", (tok, W), I32, kind="ExternalOutput")
        dbgQ = nc.dram_tensor("dbgQ", (tok, W), F32, kind="ExternalOutput")

    with TileContext(nc) as tc:
        with tc.tile_pool(name="wmat", bufs=1) as wpool, \
             tc.tile_pool(name="xp", bufs=2) as xpool, \
             tc.tile_pool(name="big", bufs=1) as bigpool, \
             tc.tile_pool(name="wqld", bufs=1) as wqldpool, \
             tc.tile_pool(name="bias", bufs=2) as biaspool, \
             tc.tile_pool(name="leaf", bufs=1) as leafpool, \
             tc.tile_pool(name="small", bufs=1) as smallpool, \
             tc.tile_pool(name="ps", bufs=3, space="PSUM") as pspool, \
             tc.tile_pool(name="pscn", bufs=2, space="PSUM") as pscnet:

            # ---------- constants ----------
            iota_cb = smallpool.tile([P, W], mybir.dt.uint16, tag="iota_cb")
            nc.gpsimd.iota(iota_cb[:], pattern=[[P, NCHUNK], [0, RUN]],
                           base=0, channel_multiplier=0)
            iota256f = smallpool.tile([P, K_MAX], F32, tag="iota256f")
            nc.gpsimd.iota(iota256f[:], pattern=[[1, K_MAX]], base=0,
                           channel_multiplier=0,
                           allow_small_or_imprecise_dtypes=True)
            ones_sb = smallpool.tile([1, P], F32, tag="ones")
            nc.vector.memset(ones_sb[:], 1.0)
            c1b_sb = smallpool.tile([P, 2], F32, tag="c1b")
            nc.sync.dma_start(c1b_sb[:], c1b[:].rearrange("(mf p) o -> p (mf o)", p=P))
            c2w_sb = smallpool.tile([P, 2], F32, tag="c2w")
            nc.sync.dma_start(c2w_sb[:], c2wT[:].rearrange("(mf p) o -> p (mf o)", p=P))
            c2b_sb = smallpool.tile([1, 1], F32, tag="c2b")
            nc.sync.dma_start(c2b_sb[:], c2b[:])
            kfm1 = smallpool.tile([P, nt], F32, tag="kfm1")   # k_float - 1 per token
            kcol = smallpool.tile([P, 1], F32, tag="kcol")

            # ---------- phase 0: complexity net ----------
            c1w_sb = wpool.tile([P, KC, 256], F32, tag="wmat")
            nc.sync.dma_start(c1w_sb[:], c1wT[:].rearrange("(kc kp) m -> kp kc m", kp=P))
            for t in range(nt):
                x_sb = xpool.tile([P, KC, P], F32, tag="xt")
                nc.sync.dma_start(
                    x_sb[:], xT[:, t * P:(t + 1) * P].rearrange("(kc kp) m -> kp kc m", kp=P))
                hT = smallpool.tile([P, 2, P], F32, tag="hT")
                for mf in range(2):
                    pcn = pscnet.tile([P, P], F32, tag="pcn")
                    for k in range(KC):
                        nc.tensor.matmul(pcn[:], c1w_sb[:, k, mf * P:(mf + 1) * P],
                                         x_sb[:, k, :], start=(k == 0), stop=(k == KC - 1))
                    nc.scalar.activation(hT[:, mf, :], pcn[:], AF.Relu,
                                         bias=c1b_sb[:, mf:mf + 1])
                pc2 = pscnet.tile([1, P], F32, tag="pc2")
                for mf in range(2):
                    nc.tensor.matmul(pc2[:], c2w_sb[:, mf:mf + 1], hT[:, mf, :],
                                     start=(mf == 0), stop=(mf == 1))
                sig = smallpool.tile([1, P], F32, tag="sig")
                nc.scalar.activation(sig[:], pc2[:], AF.Sigmoid, bias=c2b_sb[:, 0:1])
                kf_row = smallpool.tile([1, P], F32, tag="kf_row")
                nc.scalar.activation(kf_row[:], sig[:], AF.Copy,
                                     bias=float(K_MIN), scale=float(K_MAX - K_MIN))
                # transpose [1,128] -> [128,1] via DMA scatter
                nc.sync.dma_start(kcol[:, 0:1], kf_row[0:1, :])
                nc.vector.tensor_scalar(kfm1[:, t:t + 1], kcol[:, 0:1], 1.0, None,
                                        op0=OP.subtract)
                # k_int = floor(kf): RN(kf - 0.5) via 2^23 trick, then exact convert
                kvf = smallpool.tile([P, 1], F32, tag="kvf")
                nc.vector.tensor_scalar(kvf[:], kcol[:, 0:1], 0.5, 8388608.0,
                                        op0=OP.subtract, op1=OP.add)
                nc.vector.tensor_scalar(kvf[:], kvf[:], 8388608.0, None, op0=OP.subtract)
                kvi = smallpool.tile([P, 1], I32, tag="kvi")
                nc.vector.tensor_copy(kvi[:], kvf[:])
                nc.sync.dma_start(kv_o[t * P:(t + 1) * P, :], kvi[:])

            def score_phase(w_dram, b_dram, store_fn, mm_dt=F32, x_dram=None,
                            out_dt=F32):
                """Load [D, POOL] weights, per tile compute x@w.T + b into an
                SBUF tile, hand to store_fn(t, tile_ap)."""
                w_sb = wpool.tile([P, KC, POOL], mm_dt, tag="wmat")
                nc.sync.dma_start(w_sb[:], w_dram[:].rearrange("(kc kp) m -> kp kc m", kp=P))
                if x_dram is None:
                    x_dram = xT
                bh = {}
                for h in range(2):
                    bt = biaspool.tile([1, POOL // 2], F32, tag="bias")
                    bh[h] = bt
                    nc.sync.dma_start(bh[h][:], b_dram[:, h * (POOL // 2):(h + 1) * (POOL // 2)])
                for t in range(nt):
                    x_sb = xpool.tile([P, KC, P], mm_dt, tag="xt")
                    nc.sync.dma_start(
                        x_sb[:], x_dram[:, t * P:(t + 1) * P].rearrange("(kc kp) m -> kp kc m", kp=P))
                    s_sb = bigpool.tile([P, POOL], out_dt, tag="big")
                    for n in range(POOL // NCH):
                        pmm = pspool.tile([P, NCH], F32, tag="pmm")
                        for k in range(KC):
                            nc.tensor.matmul(pmm[:], x_sb[:, k, :],
                                             w_sb[:, k, n * NCH:(n + 1) * NCH],
                                             start=(k == 0), stop=False)
                        h = n // 4
                        boff = (n % 4) * NCH
                        nc.tensor.matmul(pmm[:], ones_sb[:],
                                         bh[h][0:1, boff:boff + NCH],
                                         start=False, stop=True)
                        nc.scalar.activation(s_sb[:, n * NCH:(n + 1) * NCH], pmm[:], AF.Copy)
                    store_fn(t, s_sb)

            # ---------- phase 1: wq scores -> DRAM scratch ----------
            wq_store = {}

            def wq_store_fn(t, s_sb):
                if "nodma" in _cache.get("ablate", set()):
                    wq_store[t] = None
                    return
                st = nc.sync.dma_start(wqs_s[t * P:(t + 1) * P, :], s_sb[:])
                wq_store[t] = st

            score_phase(wqwT, wqb, wq_store_fn, mm_dt=mybir.dt.bfloat16,
                        x_dram=xbfT, out_dt=mybir.dt.bfloat16)

            # ---------- phase 2: sel scores + selection ----------
            def sel_fn(t, sel_sb):
                vec = ChainedVec(nc)
                if "nodma" not in _cache.get("ablate", set()):
                    nc.sync.dma_start(scores_o[t * P:(t + 1) * P, :], sel_sb[:])

                K = leafpool.tile([P, W], U32, tag="K")     # keys (f32 bits)
                G = leafpool.tile([P, W], mybir.dt.uint16, tag="G")   # global index
                Q = leafpool.tile([P, W], mybir.dt.uint16, tag="Q")   # wq (bf16 bits)
                Kf = K[:].bitcast(F32)
                Qf = Q[:].bitcast(mybir.dt.bfloat16)
                vec.memset(Kf, -1e30)
                vec.memset(G[:], 0)
                vec.memset(Q[:], 0)
                v8 = smallpool.tile([P, 8], F32, tag="v8")
                junk = smallpool.tile([P, P], F32, tag="junk")

                last_gp = [None]
                _ab = _cache.get("ablate", set())
                for half in range(2):
                    wqlb = wqldpool.tile([P, POOL // 2], mybir.dt.bfloat16,
                                         tag="wqlb")
                    wql = wqldpool.tile([P, POOL // 2], F32, tag="wqld")
                    if wq_store[t] is not None:
                        ld = nc.sync.dma_start(
                            wqlb[:], wqs_s[t * P:(t + 1) * P,
                                           half * (POOL // 2):(half + 1) * (POOL // 2)])
                        add_dep_helper(ld.ins, wq_store[t].ins,
                                       reason="wq scratch RAW across phases")
                        nc.scalar.activation(wql[:], wqlb[:], AF.Copy)
                    for ci in range(NCHUNK // 2):
                        c = half * (NCHUNK // 2) + ci
                        chunk = sel_sb[:, c * P:(c + 1) * P]
                        wchunk = wql[:, ci * P:(ci + 1) * P]
                        asc = c % 2 == 1
                        for i in range(3):
                            if asc:
                                base = c * RUN + 8 + (2 - i) * 8
                                kslot = rev8(Kf[:, base:base + 8])
                                gslot = rev8(G[:, base:base + 8])
                                kmin = Kf[:, base:base + 1]
                            else:
                                base = c * RUN + i * 8
                                kslot = Kf[:, base:base + 8]
                                gslot = G[:, base:base + 8]
                                kmin = Kf[:, base + 7:base + 8]
                            vec.max(out=v8[:], in_=chunk)
                            vec.tensor_copy(kslot, v8[:])
                            vec.max_index(out=gslot, in_max=v8[:], in_values=chunk)
                            # ranks 21-24 keep K/G but skip the wq match:
                            # P(chunk holds >20 of top-256) ~ 3.4e-6 -> ~2
                            # tokens full-size with one zeroed pw entry.
                            nr = 4 if i == 2 else 8
                            for r in (range(0) if "stt" in _ab else range(nr)):
                                qs = base + (7 - r if asc else r)
                                vec.scalar_tensor_tensor(
                                    out=junk[:], in0=chunk, scalar=v8[:, r:r + 1],
                                    in1=wchunk, op0=OP.is_equal, op1=OP.mult,
                                    accum_out=Qf[:, qs:qs + 1])
                            if i < 2:
                                vec.match_replace(out=chunk, in_to_replace=v8[:],
                                                        in_values=chunk, imm_value=-1e30)
                vec.tensor_tensor(G[:], G[:], iota_cb[:], op=OP.add)

                # ----- bitonic merge-sort to desc-1024, payloads via XOR swap -----
                mI_t = smallpool.tile([P, W // 2], I32, tag="msk")
                x_t = smallpool.tile([P, W // 2], I32, tag="xsk")
                m16_t = smallpool.tile([P, W // 2], mybir.dt.int16, tag="msk16")
                x16_t = smallpool.tile([P, W // 2], mybir.dt.int16, tag="xsk16")
                for ki, k in enumerate([] if "merge" in _ab else [64, 128, 256, 512]):
                    if dbg and t == 0:
                        nc.sync.dma_start(dbgK[t * P:(t + 1) * P, ki * W:(ki + 1) * W], Kf)
                    j = k // 2
                    while j >= 1:
                        for par in (0, 1):
                            if k == W and par == 1:
                                break
                            if k == W:
                                wid = W if j == W // 2 else (
                                    W // 2 if j == W // 4 else W // 4)
                                ncmp = wid // 2

                                def lohi(ap, wid=wid):
                                    r = ap[:, 0:wid].rearrange("p (b two c) -> p b two c",
                                                     two=2, c=j)
                                    return r[:, :, 0, :], r[:, :, 1, :]

                                def flat(ap):
                                    return ap[:, 0:ncmp].rearrange(
                                        "p (b c) -> p b c", c=j)
                            elif k == 512 and j <= 128:
                                # truncated cleans: top-256 of each 512-run only
                                # par 0 (desc run): slots [0:256]; par 1 (asc): [768:1024]
                                base_off = 0 if par == 0 else 768
                                ncmp = 128

                                def lohi(ap, base_off=base_off):
                                    r = ap[:, base_off:base_off + 256].rearrange(
                                        "p (b two c) -> p b two c", two=2, c=j)
                                    return r[:, :, 0, :], r[:, :, 1, :]

                                def flat(ap):
                                    return ap[:, 0:ncmp].rearrange(
                                        "p (b c) -> p b c", c=j)
                            else:
                                ncmp = W // 4

                                def lohi(ap, par=par):
                                    r = ap.rearrange("p (a pp b two c) -> p a pp b two c",
                                                     pp=2, two=2, c=j,
                                                     b=k // (2 * j))
                                    return (r[:, :, par, :, 0, :],
                                            r[:, :, par, :, 1, :])

                                def flat(ap):
                                    return ap[:, 0:ncmp].rearrange(
                                        "p (a b c) -> p a b c",
                                        b=k // (2 * j), c=j)
                            mm16 = flat(m16_t[:])
                            ks = flat(x_t[:].bitcast(F32))
                            klo, khi = lohi(Kf)
                            # par 0 -> desc block: swap if hi > lo (max to lo)
                            cmp_op = OP.is_gt if par == 0 else OP.is_lt
                            vec.tensor_tensor(mm16, khi, klo, op=cmp_op)
                            vec.tensor_scalar(mm16, mm16, -1.0, None, op0=OP.mult)
                            # K: exact 3-op conditional swap via min/max
                            big_op = OP.max if par == 0 else OP.min
                            sml_op = OP.min if par == 0 else OP.max
                            vec.tensor_tensor(ks, klo, khi, op=big_op)
                            vec.tensor_tensor(khi, klo, khi, op=sml_op)
                            vec.tensor_copy(klo, ks)
                            for arr, msk, xt in (
                                    (G[:].bitcast(mybir.dt.int16), mm16, x16_t),
                                    (Q[:].bitcast(mybir.dt.int16), mm16, x16_t)):
                                xx = flat(xt[:])
                                alo, ahi = lohi(arr)
                                vec.tensor_tensor(xx, alo, ahi, op=OP.bitwise_xor)
                                vec.tensor_tensor(xx, xx, msk, op=OP.bitwise_and)
                                vec.tensor_tensor(alo, alo, xx, op=OP.bitwise_xor)
                                vec.tensor_tensor(ahi, ahi, xx, op=OP.bitwise_xor)
                        j //= 2

                # final level: cross (i, 768+i) then desc cleans on [0:256]
                fin_stages = [("cross", 256)] + [("clean", j2)
                                                 for j2 in (128, 64, 32, 16, 8, 4, 2, 1)]
                for kind, j2 in fin_stages:
                    if "merge" in _ab:
                        break
                    if kind == "cross":
                        def lohi(ap):
                            return ap[:, 0:256], ap[:, 768:1024]

                        def flat(ap):
                            return ap[:, 0:256]
                    else:
                        def lohi(ap, j2=j2):
                            r = ap[:, 0:256].rearrange("p (b two c) -> p b two c",
                                                       two=2, c=j2)
                            return r[:, :, 0, :], r[:, :, 1, :]

                        def flat(ap, j2=j2):
                            return ap[:, 0:128].rearrange("p (b c) -> p b c", c=j2)
                    mm16 = flat(m16_t[:])
                    ks = flat(x_t[:].bitcast(F32))
                    klo, khi = lohi(Kf)
                    vec.tensor_tensor(mm16, khi, klo, op=OP.is_gt)
                    vec.tensor_scalar(mm16, mm16, -1.0, None, op0=OP.mult)
                    vec.tensor_tensor(ks, klo, khi, op=OP.max)
                    vec.tensor_tensor(khi, klo, khi, op=OP.min)
                    vec.tensor_copy(klo, ks)
                    for arr, msk, xt in (
                            (G[:].bitcast(mybir.dt.int16), mm16, x16_t),
                            (Q[:].bitcast(mybir.dt.int16), mm16, x16_t)):
                        xx = flat(xt[:])
                        alo, ahi = lohi(arr)
                        vec.tensor_tensor(xx, alo, ahi, op=OP.bitwise_xor)
                        vec.tensor_tensor(xx, xx, msk, op=OP.bitwise_and)
                        vec.tensor_tensor(alo, alo, xx, op=OP.bitwise_xor)
                        vec.tensor_tensor(ahi, ahi, xx, op=OP.bitwise_xor)

                if dbg:
                    nc.sync.dma_start(dbgK[t * P:(t + 1) * P, 5 * W:6 * W], Kf)
                    nc.sync.dma_start(dbgG[t * P:(t + 1) * P, :], G[:].bitcast(I32))
                    nc.sync.dma_start(dbgQ[t * P:(t + 1) * P, :], Qf)
                # ----- outputs -----
                idx_t = smallpool.tile([P, K_MAX], I32, tag="idxt")
                vec.tensor_copy(idx_t[:], G[:, 0:K_MAX])
                nc.sync.dma_start(idx_o[t * P:(t + 1) * P, :], idx_t[:])
                pwm = smallpool.tile([P, K_MAX], F32, tag="pwm")
                vec.tensor_scalar(pwm[:], iota256f[:], kfm1[:, t:t + 1], None,
                                        op0=OP.is_le)
                vec.tensor_tensor(pwm[:], pwm[:], Qf[:, 0:K_MAX], op=OP.mult)
                nc.sync.dma_start(pw_o[t * P:(t + 1) * P, :], pwm[:])

            score_phase(selwT, selb, sel_fn)

    nc.finalize()
    return nc


def _prep_inputs(x, sel_w, sel_b, wq_w, wq_b, c1_w, c1_b, c2_w, c2_b, ncores, tokpc):
    import ml_dtypes
    xf = np.ascontiguousarray(np.asarray(x, np.float32)).reshape(-1, D)
    selwT = np.ascontiguousarray(np.asarray(sel_w, np.float32).T)
    wqwT = np.ascontiguousarray(
        np.asarray(wq_w, np.float32).T.astype(ml_dtypes.bfloat16))
    c1wT = np.ascontiguousarray(np.asarray(c1_w, np.float32).T)
    selb = np.ascontiguousarray(np.asarray(sel_b, np.float32).reshape(1, POOL))
    wqb = np.ascontiguousarray(np.asarray(wq_b, np.float32).reshape(1, POOL))
    c1bv = np.ascontiguousarray(np.asarray(c1_b, np.float32).reshape(256, 1))
    c2wT = np.ascontiguousarray(np.asarray(c2_w, np.float32).reshape(1, 256).T)
    c2bv = np.ascontiguousarray(np.asarray(c2_b, np.float32).reshape(1, 1))
    in_maps = []
    for c in range(ncores):
        shard = xf[c * tokpc:(c + 1) * tokpc]
        xTs = np.ascontiguousarray(shard.T)
        in_maps.append({
            "xT": xTs,
            "xbfT": xTs.astype(ml_dtypes.bfloat16),
            "selwT": selwT, "wqwT": wqwT, "c1wT": c1wT,
            "selb": selb, "wqb": wqb, "c1b": c1bv, "c2wT": c2wT, "c2b": c2bv,
        })
    return in_maps


def kernel(x, sel_w, sel_b, wq_w, wq_b, c1_w, c1_b, c2_w, c2_b):
    if "nc" not in _cache:
        _cache["nc"] = build(NT)
    nc = _cache["nc"]
    in_maps = _prep_inputs(x, sel_w, sel_b, wq_w, wq_b, c1_w, c1_b, c2_w, c2_b,
                           NCORES, TOK)
    from concourse.bass_utils import run_bass_kernel_spmd
    res = run_bass_kernel_spmd(nc, in_maps, core_ids=list(range(NCORES)))

    scores = np.concatenate([r["scores"] for r in res.results], axis=0)
    idx = np.concatenate([r["idx"] for r in res.results], axis=0)
    pw = np.concatenate([r["pw"] for r in res.results], axis=0)
    kv = np.concatenate([r["kv"] for r in res.results], axis=0)

    return (idx.reshape(B, S, K_MAX).astype(np.int32),
            pw.reshape(B, S, K_MAX),
            scores.reshape(B, S, POOL),
            kv.reshape(B, S).astype(np.int32))


# revision 32
# speedup vs baseline: 1.0324x; 1.0324x over previous
"""AdaptiveContextRouter Trainium2 kernel (8 NeuronCores, data-parallel).

Per core (2048 tokens): fp32 matmuls for sel/wq scores (PE, bias folded in as a
K=1 ones-row matmul), complexity net, then exact top-256 per token via
max8-leaf extraction + payload-carrying bitonic merge (key, global index and
wq value move together through XOR-swap comparators on the DVE).
"""

import sys
sys.path.insert(0, "/opt/trn_rl_repo")

import numpy as np
import concourse.bass as bass
import concourse.mybir as mybir
from concourse import bacc
from concourse.tile import TileContext
from concourse.tile_rust import add_dep_helper

F32 = mybir.dt.float32
U32 = mybir.dt.uint32
I32 = mybir.dt.int32
AF = mybir.ActivationFunctionType
OP = mybir.AluOpType

NCORES = 8
B, S, D, POOL = 4, 4096, 1024, 4096
K_MIN, K_MAX = 32, 256
TOK = B * S // NCORES          # 2048 tokens per core
P = 128
NT = TOK // P                  # 16 token tiles
KC = D // P                    # 8 contraction chunks
NCH = 512                      # psum chunk width for matmuls
RUN = 32                       # leaf run length (24 real + 8 pad)
NCHUNK = POOL // P             # 32 chunks
W = NCHUNK * RUN               # 1024 merge width

_cache = {}


class ChainedVec:
    """Proxy for nc.vector that chains every emitted DVE instruction in
    program order (the DVE is the serial bottleneck anyway)."""

    def __init__(self, nc):
        self._nc = nc
        self._lastv = None

    def __getattr__(self, name):
        return getattr(self._nc.vector, name)


def rev8(ap):
    """[128, 8] view reversed along free dim."""
    new = [list(x) for x in ap.ap]
    assert new[-1][0] == 1 and new[-1][1] == 8
    new[-1][0] = -1
    return bass.AP(ap.tensor, ap.offset + 7, new)


def strided_view(ap, off, dims):
    """AP at ap.offset+off with partition dim kept and given free [step,count] dims."""
    new = [list(ap.ap[0])] + [list(d) for d in dims]
    return bass.AP(ap.tensor, ap.offset + off, new)


def flat_view(ap, counts):
    """Contiguous AP over `ap`'s start matching a [c0, c1, ...] count structure."""
    dims = []
    stride = 1
    for c in reversed(counts):
        dims.append([stride, c])
        stride *= c
    dims.reverse()
    return bass.AP(ap.tensor, ap.offset, [list(ap.ap[0])] + dims)


def build(nt=NT):
    tok = nt * P
    nc = bacc.Bacc("TRN2", target_bir_lowering=False)

    xT = nc.dram_tensor("xT", (D, tok), F32, kind="ExternalInput")
    selwT = nc.dram_tensor("selwT", (D, POOL), F32, kind="ExternalInput")
    wqwT = nc.dram_tensor("wqwT", (D, POOL), mybir.dt.bfloat16, kind="ExternalInput")
    xbfT = nc.dram_tensor("xbfT", (D, tok), mybir.dt.bfloat16, kind="ExternalInput")
    c1wT = nc.dram_tensor("c1wT", (D, 256), F32, kind="ExternalInput")
    selb = nc.dram_tensor("selb", (1, POOL), F32, kind="ExternalInput")
    wqb = nc.dram_tensor("wqb", (1, POOL), F32, kind="ExternalInput")
    c1b = nc.dram_tensor("c1b", (256, 1), F32, kind="ExternalInput")
    c2wT = nc.dram_tensor("c2wT", (256, 1), F32, kind="ExternalInput")
    c2b = nc.dram_tensor("c2b", (1, 1), F32, kind="ExternalInput")

    scores_o = nc.dram_tensor("scores", (tok, POOL), F32, kind="ExternalOutput")
    idx_o = nc.dram_tensor("idx", (tok, K_MAX), I32, kind="ExternalOutput")
    pw_o = nc.dram_tensor("pw", (tok, K_MAX), F32, kind="ExternalOutput")
    kv_o = nc.dram_tensor("kv", (tok, 1), I32, kind="ExternalOutput")

    wqs_s = nc.dram_tensor("wqs_s", (tok, POOL), mybir.dt.bfloat16, kind="Internal")
    dbg = _cache.get("dbg")
    if dbg:
        dbgK = nc.dram_tensor("dbgK", (tok, 6 * W), F32, kind="ExternalOutput")
        dbgG = nc.dram_tensor("dbgG", (tok, W), I32, kind="ExternalOutput")
        dbgQ = nc.dram_tensor("dbgQ", (tok, W), F32, kind="ExternalOutput")

    with TileContext(nc) as tc:
        with tc.tile_pool(name="wmat", bufs=1) as wpool, \
             tc.tile_pool(name="xp", bufs=2) as xpool, \
             tc.tile_pool(name="big", bufs=1) as bigpool, \
             tc.tile_pool(name="wqld", bufs=1) as wqldpool, \
             tc.tile_pool(name="bias", bufs=2) as biaspool, \
             tc.tile_pool(name="leaf", bufs=1) as leafpool, \
             tc.tile_pool(name="small", bufs=1) as smallpool, \
             tc.tile_pool(name="ps", bufs=3, space="PSUM") as pspool, \
             tc.tile_pool(name="pscn", bufs=2, space="PSUM") as pscnet:

            # ---------- constants ----------
            iota_cb = smallpool.tile([P, W], mybir.dt.uint16, tag="iota_cb")
            nc.gpsimd.iota(iota_cb[:], pattern=[[P, NCHUNK], [0, RUN]],
                           base=0, channel_multiplier=0)
            iota256f = smallpool.tile([P, K_MAX], F32, tag="iota256f")
            nc.gpsimd.iota(iota256f[:], pattern=[[1, K_MAX]], base=0,
                           channel_multiplier=0,
                           allow_small_or_imprecise_dtypes=True)
            ones_sb = smallpool.tile([1, P], F32, tag="ones")
            nc.vector.memset(ones_sb[:], 1.0)
            c1b_sb = smallpool.tile([P, 2], F32, tag="c1b")
            nc.sync.dma_start(c1b_sb[:], c1b[:].rearrange("(mf p) o -> p (mf o)", p=P))
            c2w_sb = smallpool.tile([P, 2], F32, tag="c2w")
            nc.sync.dma_start(c2w_sb[:], c2wT[:].rearrange("(mf p) o -> p (mf o)", p=P))
            c2b_sb = smallpool.tile([1, 1], F32, tag="c2b")
            nc.sync.dma_start(c2b_sb[:], c2b[:])
            kfm1 = smallpool.tile([P, nt], F32, tag="kfm1")   # k_float - 1 per token
            kcol = smallpool.tile([P, 1], F32, tag="kcol")

            # ---------- phase 0: complexity net ----------
            c1w_sb = wpool.tile([P, KC, 256], F32, tag="wmat")
            nc.sync.dma_start(c1w_sb[:], c1wT[:].rearrange("(kc kp) m -> kp kc m", kp=P))
            for t in range(nt):
                x_sb = xpool.tile([P, KC, P], F32, tag="xt")
                nc.sync.dma_start(
                    x_sb[:], xT[:, t * P:(t + 1) * P].rearrange("(kc kp) m -> kp kc m", kp=P))
                hT = smallpool.tile([P, 2, P], F32, tag="hT")
                for mf in range(2):
                    pcn = pscnet.tile([P, P], F32, tag="pcn")
                    for k in range(KC):
                        nc.tensor.matmul(pcn[:], c1w_sb[:, k, mf * P:(mf + 1) * P],
                                         x_sb[:, k, :], start=(k == 0), stop=(k == KC - 1))
                    nc.scalar.activation(hT[:, mf, :], pcn[:], AF.Relu,
                                         bias=c1b_sb[:, mf:mf + 1])
                pc2 = pscnet.tile([1, P], F32, tag="pc2")
                for mf in range(2):
                    nc.tensor.matmul(pc2[:], c2w_sb[:, mf:mf + 1], hT[:, mf, :],
                                     start=(mf == 0), stop=(mf == 1))
                sig = smallpool.tile([1, P], F32, tag="sig")
                nc.scalar.activation(sig[:], pc2[:], AF.Sigmoid, bias=c2b_sb[:, 0:1])
                kf_row = smallpool.tile([1, P], F32, tag="kf_row")
                nc.scalar.activation(kf_row[:], sig[:], AF.Copy,
                                     bias=float(K_MIN), scale=float(K_MAX - K_MIN))
                # transpose [1,128] -> [128,1] via DMA scatter
                nc.sync.dma_start(kcol[:, 0:1], kf_row[0:1, :])
                nc.vector.tensor_scalar(kfm1[:, t:t + 1], kcol[:, 0:1], 1.0, None,
                                        op0=OP.subtract)
                # k_int = floor(kf): RN(kf - 0.5) via 2^23 trick, then exact convert
                kvf = smallpool.tile([P, 1], F32, tag="kvf")
                nc.vector.tensor_scalar(kvf[:], kcol[:, 0:1], 0.5, 8388608.0,
                                        op0=OP.subtract, op1=OP.add)
                nc.vector.tensor_scalar(kvf[:], kvf[:], 8388608.0, None, op0=OP.subtract)
                kvi = smallpool.tile([P, 1], I32, tag="kvi")
                nc.vector.tensor_copy(kvi[:], kvf[:])
                nc.sync.dma_start(kv_o[t * P:(t + 1) * P, :], kvi[:])

            def score_phase(w_dram, b_dram, store_fn, mm_dt=F32, x_dram=None,
                            out_dt=F32):
                """Load [D, POOL] weights, per tile compute x@w.T + b into an
                SBUF tile, hand to store_fn(t, tile_ap)."""
                w_sb = wpool.tile([P, KC, POOL], mm_dt, tag="wmat")
                nc.sync.dma_start(w_sb[:], w_dram[:].rearrange("(kc kp) m -> kp kc m", kp=P))
                if x_dram is None:
                    x_dram = xT
                bh = {}
                for h in range(2):
                    bt = biaspool.tile([1, POOL // 2], F32, tag="bias")
                    bh[h] = bt
                    nc.sync.dma_start(bh[h][:], b_dram[:, h * (POOL // 2):(h + 1) * (POOL // 2)])
                for t in range(nt):
                    x_sb = xpool.tile([P, KC, P], mm_dt, tag="xt")
                    nc.sync.dma_start(
                        x_sb[:], x_dram[:, t * P:(t + 1) * P].rearrange("(kc kp) m -> kp kc m", kp=P))
                    s_sb = bigpool.tile([P, POOL], out_dt, tag="big")
                    for n in range(POOL // NCH):
                        pmm = pspool.tile([P, NCH], F32, tag="pmm")
                        for k in range(KC):
                            nc.tensor.matmul(pmm[:], x_sb[:, k, :],
                                             w_sb[:, k, n * NCH:(n + 1) * NCH],
                                             start=(k == 0), stop=False)
                        h = n // 4
                        boff = (n % 4) * NCH
                        nc.tensor.matmul(pmm[:], ones_sb[:],
                                         bh[h][0:1, boff:boff + NCH],
                                         start=False, stop=True)
                        nc.scalar.activation(s_sb[:, n * NCH:(n + 1) * NCH], pmm[:], AF.Copy)
                    store_fn(t, s_sb)

            # ---------- phase 1: wq scores -> DRAM scratch ----------
            wq_store = {}

            def wq_store_fn(t, s_sb):
                if "nodma" in _cache.get("ablate", set()):
                    wq_store[t] = None
                    return
                st = nc.sync.dma_start(wqs_s[t * P:(t + 1) * P, :], s_sb[:])
                wq_store[t] = st

            score_phase(wqwT, wqb, wq_store_fn, mm_dt=mybir.dt.bfloat16,
                        x_dram=xbfT, out_dt=mybir.dt.bfloat16)

            # ---------- phase 2: sel scores + selection ----------
            def sel_fn(t, sel_sb):
                vec = ChainedVec(nc)
                if "nodma" not in _cache.get("ablate", set()):
                    nc.sync.dma_start(scores_o[t * P:(t + 1) * P, :], sel_sb[:])

                K = leafpool.tile([P, W], U32, tag="K")     # keys (f32 bits)
                G = leafpool.tile([P, W], mybir.dt.uint16, tag="G")   # global index
                Q = leafpool.tile([P, W], mybir.dt.uint16, tag="Q")   # wq (bf16 bits)
                Kf = K[:].bitcast(F32)
                Qf = Q[:].bitcast(mybir.dt.bfloat16)
                vec.memset(Kf, -1e30)
                vec.memset(G[:], 0)
                vec.memset(Q[:], 0)
                v8 = smallpool.tile([P, 8], F32, tag="v8")
                junk = smallpool.tile([P, P], F32, tag="junk")

                last_gp = [None]
                _ab = _cache.get("ablate", set())
                for half in range(2):
                    wqlb = wqldpool.tile([P, POOL // 2], mybir.dt.bfloat16,
                                         tag="wqlb")
                    wql = wqldpool.tile([P, POOL // 2], F32, tag="wqld")
                    if wq_store[t] is not None:
                        ld = nc.sync.dma_start(
                            wqlb[:], wqs_s[t * P:(t + 1) * P,
                                           half * (POOL // 2):(half + 1) * (POOL // 2)])
                        add_dep_helper(ld.ins, wq_store[t].ins,
                                       reason="wq scratch RAW across phases")
                        nc.scalar.activation(wql[:], wqlb[:], AF.Copy)
                    for ci in range(NCHUNK // 2):
                        c = half * (NCHUNK // 2) + ci
                        chunk = sel_sb[:, c * P:(c + 1) * P]
                        wchunk = wql[:, ci * P:(ci + 1) * P]
                        asc = c % 2 == 1
                        for i in range(3):
                            if asc:
                                base = c * RUN + 8 + (2 - i) * 8
                                kslot = rev8(Kf[:, base:base + 8])
                                gslot = rev8(G[:, base:base + 8])
                                kmin = Kf[:, base:base + 1]
                            else:
                                base = c * RUN + i * 8
                                kslot = Kf[:, base:base + 8]
                                gslot = G[:, base:base + 8]
                                kmin = Kf[:, base + 7:base + 8]
                            vec.max(out=kslot, in_=chunk)
                            vec.max_index(out=gslot, in_max=kslot, in_values=chunk)
                            # ranks 21-24 keep K/G but skip the wq match:
                            # P(chunk holds >20 of top-256) ~ 3.4e-6 -> ~2
                            # tokens full-size with one zeroed pw entry.
                            nr = 4 if i == 2 else 8
                            for r in (range(0) if "stt" in _ab else range(nr)):
                                qs = base + (7 - r if asc else r)
                                vec.scalar_tensor_tensor(
                                    out=junk[:], in0=chunk,
                                    scalar=Kf[:, qs:qs + 1],
                                    in1=wchunk, op0=OP.is_equal, op1=OP.mult,
                                    accum_out=Qf[:, qs:qs + 1])
                            if i < 2:
                                vec.match_replace(out=chunk, in_to_replace=kslot,
                                                        in_values=chunk, imm_value=-1e30)
                vec.tensor_tensor(G[:], G[:], iota_cb[:], op=OP.add)

                # ----- bitonic merge-sort to desc-1024, payloads via XOR swap -----
                mI_t = smallpool.tile([P, W // 2], I32, tag="msk")
                x_t = smallpool.tile([P, W // 2], I32, tag="xsk")
                m16_t = smallpool.tile([P, W // 2], mybir.dt.int16, tag="msk16")
                x16_t = smallpool.tile([P, W // 2], mybir.dt.int16, tag="xsk16")
                for ki, k in enumerate([] if "merge" in _ab else [64, 128, 256, 512]):
                    if dbg and t == 0:
                        nc.sync.dma_start(dbgK[t * P:(t + 1) * P, ki * W:(ki + 1) * W], Kf)
                    j = k // 2
                    while j >= 1:
                        for par in (0, 1):
                            if k == W and par == 1:
                                break
                            if k == W:
                                wid = W if j == W // 2 else (
                                    W // 2 if j == W // 4 else W // 4)
                                ncmp = wid // 2

                                def lohi(ap, wid=wid):
                                    r = ap[:, 0:wid].rearrange("p (b two c) -> p b two c",
                                                     two=2, c=j)
                                    return r[:, :, 0, :], r[:, :, 1, :]

                                def flat(ap):
                                    return ap[:, 0:ncmp].rearrange(
                                        "p (b c) -> p b c", c=j)
                            elif k == 512 and j <= 128:
                                # truncated cleans: top-256 of each 512-run only
                                # par 0 (desc run): slots [0:256]; par 1 (asc): [768:1024]
                                base_off = 0 if par == 0 else 768
                                ncmp = 128

                                def lohi(ap, base_off=base_off):
                                    r = ap[:, base_off:base_off + 256].rearrange(
                                        "p (b two c) -> p b two c", two=2, c=j)
                                    return r[:, :, 0, :], r[:, :, 1, :]

                                def flat(ap):
                                    return ap[:, 0:ncmp].rearrange(
                                        "p (b c) -> p b c", c=j)
                            else:
                                ncmp = W // 4

                                def lohi(ap, par=par):
                                    r = ap.rearrange("p (a pp b two c) -> p a pp b two c",
                                                     pp=2, two=2, c=j,
                                                     b=k // (2 * j))
                                    return (r[:, :, par, :, 0, :],
                                            r[:, :, par, :, 1, :])

                                def flat(ap):
                                    return ap[:, 0:ncmp].rearrange(
                                        "p (a b c) -> p a b c",
                                        b=k // (2 * j), c=j)
                            mm16 = flat(m16_t[:])
                            ks = flat(x_t[:].bitcast(F32))
                            klo, khi = lohi(Kf)
                            # par 0 -> desc block: swap if hi > lo (max to lo)
                            cmp_op = OP.is_gt if par == 0 else OP.is_lt
                            vec.tensor_tensor(mm16, khi, klo, op=cmp_op)
                            vec.tensor_scalar(mm16, mm16, -1.0, None, op0=OP.mult)
                            # K: exact 3-op conditional swap via min/max
                            big_op = OP.max if par == 0 else OP.min
                            sml_op = OP.min if par == 0 else OP.max
                            vec.tensor_tensor(ks, klo, khi, op=big_op)
                            vec.tensor_tensor(khi, klo, khi, op=sml_op)
                            vec.tensor_copy(klo, ks)
                            for arr, msk, xt in (
                                    (G[:].bitcast(mybir.dt.int16), mm16, x16_t),
                                    (Q[:].bitcast(mybir.dt.int16), mm16, x16_t)):
                                xx = flat(xt[:])
                                alo, ahi = lohi(arr)
                                vec.tensor_tensor(xx, alo, ahi, op=OP.bitwise_xor)
                                vec.tensor_tensor(xx, xx, msk, op=OP.bitwise_and)
                                vec.tensor_tensor(alo, alo, xx, op=OP.bitwise_xor)
                                vec.tensor_tensor(ahi, ahi, xx, op=OP.bitwise_xor)
                        j //= 2

                # final level: cross (i, 768+i) then desc cleans on [0:256]
                fin_stages = [("cross", 256)] + [("clean", j2)
                                                 for j2 in (128, 64, 32, 16, 8, 4, 2, 1)]
                for kind, j2 in fin_stages:
                    if "merge" in _ab:
                        break
                    if kind == "cross":
                        def lohi(ap):
                            return ap[:, 0:256], ap[:, 768:1024]

                        def flat(ap):
                            return ap[:, 0:256]
                    else:
                        def lohi(ap, j2=j2):
                            r = ap[:, 0:256].rearrange("p (b two c) -> p b two c",
                                                       two=2, c=j2)
                            return r[:, :, 0, :], r[:, :, 1, :]

                        def flat(ap, j2=j2):
                            return ap[:, 0:128].rearrange("p (b c) -> p b c", c=j2)
                    mm16 = flat(m16_t[:])
                    ks = flat(x_t[:].bitcast(F32))
                    klo, khi = lohi(Kf)
                    vec.tensor_tensor(mm16, khi, klo, op=OP.is_gt)
                    vec.tensor_scalar(mm16, mm16, -1.0, None, op0=OP.mult)
                    vec.tensor_tensor(ks, klo, khi, op=OP.max)
                    vec.tensor_tensor(khi, klo, khi, op=OP.min)
                    vec.tensor_copy(klo, ks)
                    for arr, msk, xt in (
                            (G[:].bitcast(mybir.dt.int16), mm16, x16_t),
                            (Q[:].bitcast(mybir.dt.int16), mm16, x16_t)):
                        xx = flat(xt[:])
                        alo, ahi = lohi(arr)
                        vec.tensor_tensor(xx, alo, ahi, op=OP.bitwise_xor)
                        vec.tensor_tensor(xx, xx, msk, op=OP.bitwise_and)
                        vec.tensor_tensor(alo, alo, xx, op=OP.bitwise_xor)
                        vec.tensor_tensor(ahi, ahi, xx, op=OP.bitwise_xor)

                if dbg:
                    nc.sync.dma_start(dbgK[t * P:(t + 1) * P, 5 * W:6 * W], Kf)
                    nc.sync.dma_start(dbgG[t * P:(t + 1) * P, :], G[:].bitcast(I32))
                    nc.sync.dma_start(dbgQ[t * P:(t + 1) * P, :], Qf)
                # ----- outputs -----
                idx_t = smallpool.tile([P, K_MAX], I32, tag="idxt")
                vec.tensor_copy(idx_t[:], G[:, 0:K_MAX])
                nc.sync.dma_start(idx_o[t * P:(t + 1) * P, :], idx_t[:])
                pwm = smallpool.tile([P, K_MAX], F32, tag="pwm")
                vec.tensor_scalar(pwm[:], iota256f[:], kfm1[:, t:t + 1], None,
                                        op0=OP.is_le)
                vec.tensor_tensor(pwm[:], pwm[:], Qf[:, 0:K_MAX], op=OP.mult)
                nc.sync.dma_start(pw_o[t * P:(t + 1) * P, :], pwm[:])

            score_phase(selwT, selb, sel_fn)

    nc.finalize()
    return nc


def _prep_inputs(x, sel_w, sel_b, wq_w, wq_b, c1_w, c1_b, c2_w, c2_b, ncores, tokpc):
    import ml_dtypes
    xf = np.ascontiguousarray(np.asarray(x, np.float32)).reshape(-1, D)
    selwT = np.ascontiguousarray(np.asarray(sel_w, np.float32).T)
    wqwT = np.ascontiguousarray(
        np.asarray(wq_w, np.float32).T.astype(ml_dtypes.bfloat16))
    c1wT = np.ascontiguousarray(np.asarray(c1_w, np.float32).T)
    selb = np.ascontiguousarray(np.asarray(sel_b, np.float32).reshape(1, POOL))
    wqb = np.ascontiguousarray(np.asarray(wq_b, np.float32).reshape(1, POOL))
    c1bv = np.ascontiguousarray(np.asarray(c1_b, np.float32).reshape(256, 1))
    c2wT = np.ascontiguousarray(np.asarray(c2_w, np.float32).reshape(1, 256).T)
    c2bv = np.ascontiguousarray(np.asarray(c2_b, np.float32).reshape(1, 1))
    in_maps = []
    for c in range(ncores):
        shard = xf[c * tokpc:(c + 1) * tokpc]
        xTs = np.ascontiguousarray(shard.T)
        in_maps.append({
            "xT": xTs,
            "xbfT": xTs.astype(ml_dtypes.bfloat16),
            "selwT": selwT, "wqwT": wqwT, "c1wT": c1wT,
            "selb": selb, "wqb": wqb, "c1b": c1bv, "c2wT": c2wT, "c2b": c2bv,
        })
    return in_maps


def kernel(x, sel_w, sel_b, wq_w, wq_b, c1_w, c1_b, c2_w, c2_b):
    if "nc" not in _cache:
        _cache["nc"] = build(NT)
    nc = _cache["nc"]
    in_maps = _prep_inputs(x, sel_w, sel_b, wq_w, wq_b, c1_w, c1_b, c2_w, c2_b,
                           NCORES, TOK)
    from concourse.bass_utils import run_bass_kernel_spmd
    res = run_bass_kernel_spmd(nc, in_maps, core_ids=list(range(NCORES)))

    scores = np.concatenate([r["scores"] for r in res.results], axis=0)
    idx = np.concatenate([r["idx"] for r in res.results], axis=0)
    pw = np.concatenate([r["pw"] for r in res.results], axis=0)
    kv = np.concatenate([r["kv"] for r in res.results], axis=0)

    return (idx.reshape(B, S, K_MAX).astype(np.int32),
            pw.reshape(B, S, K_MAX),
            scores.reshape(B, S, POOL),
            kv.reshape(B, S).astype(np.int32))
